# revision 1
# baseline (speedup 1.0000x reference)
"""Two-layer GraphConv (gather + segment-mean + linear + ReLU) x2 + sigmoid head,
distributed over 8 NeuronCores.

Sharding: destination nodes are partitioned across the 8 cores (12.5k each).
Host-side prep (pure index work): each core's edges are bucketed by
(src-chunk-of-25k, dst), each (chunk x dst-tile-of-128) run is padded to a
multiple of 128 with sentinel edges so all 8 cores share one SPMD program.

On device, per layer:
  - dma_gather fetches 256B source rows (int16 chunk-local indices)
  - one-hot matrices are built on the vector engine by comparing an iota
    constant against per-edge relative-dst values
  - TensorE matmuls (lhsT=one-hot, rhs=gathered msgs) segment-sum into PSUM,
    accumulated per dst-tile into an SBUF accumulator
  - scale by 1/deg, PE-transpose, fused W+bias matmuls, ReLU
  - AllGather of x1 between the layers
  - layer-2 tail: ReLU with accumulated row-sum, sigmoid(scale*s+bias)
"""

import os
import sys

for _p in ("/opt/trn_rl_repo", "/opt/pypackages"):
    if _p not in sys.path and os.path.isdir(_p):
        sys.path.insert(0, _p)

import numpy as np

from concourse import bacc, bass, mybir, tile
from concourse.bass_utils import run_bass_kernel_spmd

F32 = mybir.dt.float32
I16 = mybir.dt.int16

TILE = 128


def _cdiv(a, b):
    return (a + b - 1) // b


class Cfg:
    def __init__(self, N=100000, D=64, C=8, CH=25000, BSZ=1024, no_cc=False):
        self.no_cc = no_cc
        assert N % C == 0 and N % CH == 0
        assert CH <= 32768  # int16 gather indices
        assert BSZ % 128 == 0
        self.N, self.D, self.C, self.CH, self.BSZ = N, D, C, CH, BSZ
        self.NDST = N // C
        self.NT = _cdiv(self.NDST, TILE)
        self.NP = N // CH
        self.D2 = 32  # layer-2 output width


def plan_edges(edge_src, edge_dst, cfg):
    """Bucket/sort/pad edges per core; all cores share the quota structure."""
    src = np.asarray(edge_src).astype(np.int64)
    dst = np.asarray(edge_dst).astype(np.int64)
    C, CH, NT, NP, NDST = cfg.C, cfg.CH, cfg.NT, cfg.NP, cfg.NDST

    percore = []
    counts = []
    for c in range(C):
        m = (dst // NDST) == c
        s = src[m]
        dl = dst[m] - c * NDST
        p = s // CH
        o = np.lexsort((dl, p))
        s, dl, p = s[o], dl[o], p[o]
        t = dl >> 7
        cnt = np.bincount(p * NT + t, minlength=NP * NT).reshape(NP, NT)
        percore.append((s, dl, p, t))
        counts.append(cnt)

    quota = np.maximum.reduce(counts)
    quota = (quota + TILE - 1) // TILE * TILE  # pad runs to group multiples
    qflat = quota.reshape(-1)
    offs = np.concatenate([[0], np.cumsum(qflat)])
    T = int(offs[-1])
    offs_flat = offs[:-1].reshape(NP, NT)
    Lp = quota.sum(axis=1)

    # batches: per pass, chunks of BSZ stream positions (last one ragged)
    batches = []  # list of (pass, global_offset, nb)
    pass_base = np.concatenate([[0], np.cumsum(Lp)])
    for p in range(NP):
        off = 0
        while off < Lp[p]:
            nb = int(min(cfg.BSZ, Lp[p] - off))
            batches.append((p, int(pass_base[p] + off), nb))
            off += nb

    # group -> tile map + run boundary flags (shared across cores)
    NG = T // TILE
    group_tile = np.zeros(NG, np.int32)
    group_first = np.zeros(NG, bool)
    group_last = np.zeros(NG, bool)
    for p in range(NP):
        for t in range(NT):
            q = quota[p, t]
            if q == 0:
                continue
            g0 = offs_flat[p, t] // TILE
            g1 = g0 + q // TILE
            group_tile[g0:g1] = t
            group_first[g0] = True
            group_last[g1 - 1] = True

    per_core_arrays = []
    for c in range(C):
        s, dl, p, t = percore[c]
        key = p * NT + t
        first = np.searchsorted(key, np.arange(NP * NT), side="left")
        rank = np.arange(len(key)) - first[key]
        pos = offs_flat[p, t] + rank
        srcl = np.zeros(T, np.int16)
        drel = np.full(T, 200.0, np.float32)  # sentinel: never matches iota 0..127
        srcl[pos] = (s - p * CH).astype(np.int16)
        drel[pos] = (dl - (t << 7)).astype(np.float32)

        deg = np.bincount(dl, minlength=NDST).astype(np.float32)
        deg = np.maximum(deg, 1.0)
        degp = np.ones(NT * TILE, np.float32)
        degp[:NDST] = deg
        deg_arr = degp.reshape(NT, TILE).T.copy()  # [128, NT]

        idxw = np.tile(srcl.reshape(T // 16, 16).T, (8, 1)).copy()  # [128, T/16]
        drw = drel.reshape(T // TILE, TILE).T.copy()  # [128, T/128]
        per_core_arrays.append(dict(idxs=idxw, drel=drw, deg=deg_arr))

    structure = dict(
        T=T,
        NG=NG,
        batches=tuple(batches),
        group_tile=tuple(int(v) for v in group_tile),
        group_first=tuple(bool(v) for v in group_first),
        group_last=tuple(bool(v) for v in group_last),
    )
    return structure, per_core_arrays


def build_program(cfg, structure):
    N, D, C, CH, NT, NP = cfg.N, cfg.D, cfg.C, cfg.CH, cfg.NT, cfg.NP
    D2 = cfg.D2
    NDST = cfg.NDST
    T = structure["T"]
    batches = structure["batches"]
    group_tile = structure["group_tile"]
    group_first = structure["group_first"]
    group_last = structure["group_last"]
    OH_GROUPS = 16  # one-hot groups built per DVE op
    Relu = mybir.ActivationFunctionType.Relu
    Copy = mybir.ActivationFunctionType.Copy
    Sigmoid = mybir.ActivationFunctionType.Sigmoid

    nc = bacc.Bacc(None, target_bir_lowering=False, num_swdge_queues=4)
    x0 = nc.dram_tensor("x0", [N, D], F32, kind="ExternalInput")
    idxs_d = nc.dram_tensor("idxs", [128, T // 16], I16, kind="ExternalInput")
    drel_d = nc.dram_tensor("drel", [128, T // TILE], F32, kind="ExternalInput")
    deg_d = nc.dram_tensor("deg", [128, NT], F32, kind="ExternalInput")
    w1_d = nc.dram_tensor("w1", [D, D], F32, kind="ExternalInput")
    b1_d = nc.dram_tensor("b1", [1, D], F32, kind="ExternalInput")
    w2_d = nc.dram_tensor("w2", [D, D2], F32, kind="ExternalInput")
    b2_d = nc.dram_tensor("b2", [1, D2], F32, kind="ExternalInput")
    wdbd_d = nc.dram_tensor("wdbd", [1, 2], F32, kind="ExternalInput")
    iota_d = nc.dram_tensor("iota", [128, OH_GROUPS * TILE], F32, kind="ExternalInput")
    ident_d = nc.dram_tensor("ident", [128, 128], F32, kind="ExternalInput")
    ones_d = nc.dram_tensor("ones1", [1, 128], F32, kind="ExternalInput")
    outp = nc.dram_tensor("out", [NDST, 1], F32, kind="ExternalOutput")
    x1loc = nc.dram_tensor("x1loc", [NDST, D], F32)
    x1full = nc.dram_tensor("x1full", [N, D], F32, addr_space="Shared")

    NFULL = NDST // TILE  # full dst tiles
    REM = NDST - NFULL * TILE  # lanes in the last (partial) tile, 0 if none

    with tile.TileContext(nc) as tc:
        with (
            tc.tile_pool(name="const", bufs=1) as cp,
            tc.tile_pool(name="work", bufs=4) as wp,
            tc.tile_pool(name="ohp", bufs=4) as ohp,
            tc.tile_pool(name="psacc", bufs=4, space="PSUM") as ps_acc,
            tc.tile_pool(name="pst", bufs=2, space="PSUM") as ps_t,
            tc.tile_pool(name="psm", bufs=2, space="PSUM") as ps_m,
        ):
            # ---- constants into SBUF ----
            iota_sb = cp.tile([128, OH_GROUPS * TILE], F32)
            nc.sync.dma_start(iota_sb[:], iota_d[:, :])
            ident_sb = cp.tile([128, 128], F32)
            nc.sync.dma_start(ident_sb[:], ident_d[:, :])
            ones_sb = cp.tile([1, 128], F32)
            nc.sync.dma_start(ones_sb[:], ones_d[:, :])
            w1_sb = cp.tile([D, D], F32)
            nc.sync.dma_start(w1_sb[:], w1_d[:, :])
            b1_sb = cp.tile([1, D], F32)
            nc.sync.dma_start(b1_sb[:], b1_d[:, :])
            w2_sb = cp.tile([D, D2], F32)
            nc.sync.dma_start(w2_sb[:], w2_d[:, :])
            b2_sb = cp.tile([1, D2], F32)
            nc.sync.dma_start(b2_sb[:], b2_d[:, :])
            wdbd_sb = cp.tile([1, 2], F32)
            nc.sync.dma_start(wdbd_sb[:], wdbd_d[:, :])
            deg_sb = cp.tile([128, NT], F32)
            nc.sync.dma_start(deg_sb[:], deg_d[:, :])

            rdeg = cp.tile([128, NT], F32)
            nc.vector.reciprocal(rdeg[:], deg_sb[:])

            def pe_fence(*aps):
                for ap in aps:
                    with tc.tile_critical():
                        nop = nc.tensor.nop(hint="dep").ins
                        nop.ins = [nc.tensor.lower_ap(ap)]

            # broadcast Wd/32 and bd across partitions via a K=1 matmul
            pe_fence(ones_sb[:], wdbd_sb[:])
            wb_ps = ps_m.tile([128, 64], F32, tag="mm", name="wb_ps")
            nc.tensor.matmul(wb_ps[:, :2], lhsT=ones_sb[:], rhs=wdbd_sb[:],
                             start=True, stop=True)
            wb_rep = cp.tile([128, 2], F32)
            nc.scalar.activation(wb_rep[:], wb_ps[:, :2], Copy)
            nc.vector.tensor_scalar_mul(wb_rep[:, 0:1], wb_rep[:, 0:1], 1.0 / 32.0)

            agg = cp.tile([128, NT * D], F32)
            x1sb = cp.tile([128, NT * D], F32)
            res = cp.tile([128, NT], F32)

            def do_layer(table, last):
                nc.vector.memset(agg[:], 0.0)
                cur_ps = [None]

                for bi, (p, boff, nb) in enumerate(batches):
                    ncol = nb // TILE
                    idx_t = wp.tile([128, nb // 16], I16, tag="idx")
                    nc.sync.dma_start(
                        idx_t[:], idxs_d[:, boff // 16:(boff + nb) // 16])
                    dr_t = wp.tile([128, ncol], F32, tag="dr")
                    nc.sync.dma_start(
                        dr_t[:], drel_d[:, boff // TILE:(boff + nb) // TILE])
                    msgs = wp.tile([128, ncol * D], F32, tag="msgs")
                    msgs3 = msgs[:].rearrange("p (c f) -> p c f", f=D)
                    nc.gpsimd.dma_gather(
                        msgs3,
                        table[p * CH:(p + 1) * CH, :],
                        idx_t[:],
                        nb,
                        nb,
                        D,
                        queue_num=bi % 4,
                    )
                    nsub = _cdiv(ncol, OH_GROUPS)
                    for sc in range(nsub):
                        gcols = min(OH_GROUPS, ncol - sc * OH_GROUPS)
                        m = gcols * TILE
                        oh = ohp.tile([128, OH_GROUPS * TILE], F32, tag="oh")
                        in1 = (
                            dr_t[:, sc * OH_GROUPS: sc * OH_GROUPS + gcols]
                            .rearrange("p (g o) -> p g o", o=1)
                            .to_broadcast([128, gcols, TILE])
                        )
                        nc.vector.tensor_tensor(
                            out=oh[:, :m],
                            in0=iota_sb[:, :m],
                            in1=in1,
                            op=mybir.AluOpType.is_equal,
                        )
                        pe_fence(oh[:, :m], msgs[:])
                        for g in range(gcols):
                            gg = boff // TILE + sc * OH_GROUPS + g
                            t = group_tile[gg]
                            if group_first[gg]:
                                cur_ps[0] = ps_acc.tile(
                                    [128, D], F32, tag="acc", name="accps")
                            nc.tensor.matmul(
                                cur_ps[0][:],
                                lhsT=oh[:, g * TILE:(g + 1) * TILE],
                                rhs=msgs[:, (sc * OH_GROUPS + g) * D:
                                         (sc * OH_GROUPS + g + 1) * D],
                                start=group_first[gg],
                                stop=group_last[gg],
                            )
                            if group_last[gg]:
                                nc.vector.tensor_add(
                                    agg[:, t * D:(t + 1) * D],
                                    agg[:, t * D:(t + 1) * D],
                                    cur_ps[0][:],
                                )

                for t in range(NT):
                    scaled = wp.tile([128, D], F32, tag="scaled")
                    nc.vector.tensor_scalar_mul(
                        scaled[:], agg[:, t * D:(t + 1) * D], rdeg[:, t:t + 1])
                    pe_fence(scaled[:], ident_sb[:])
                    tps = ps_t.tile([D, 128], F32, tag="tps")
                    nc.tensor.transpose(tps[:], scaled[:], ident_sb[:])
                    aggT = wp.tile([D, 128], F32, tag="aggT")
                    nc.scalar.activation(aggT[:], tps[:], Copy)
                    if not last:
                        pe_fence(aggT[:], w1_sb[:], ones_sb[:], b1_sb[:])
                        x1ps = ps_m.tile([128, D], F32, tag="mm", name="x1ps")
                        nc.tensor.matmul(x1ps[:], lhsT=aggT[:], rhs=w1_sb[:],
                                         start=True, stop=False)
                        nc.tensor.matmul(x1ps[:], lhsT=ones_sb[:], rhs=b1_sb[:],
                                         start=False, stop=True)
                        nc.scalar.activation(
                            x1sb[:, t * D:(t + 1) * D], x1ps[:], Relu)
                    else:
                        pe_fence(aggT[:], w2_sb[:], ones_sb[:], b2_sb[:])
                        x2ps = ps_m.tile([128, D], F32, tag="mm", name="x2ps")
                        nc.tensor.matmul(x2ps[:, :D2], lhsT=aggT[:], rhs=w2_sb[:],
                                         start=True, stop=False)
                        nc.tensor.matmul(x2ps[:, :D2], lhsT=ones_sb[:], rhs=b2_sb[:],
                                         start=False, stop=True)
                        x2sb = wp.tile([128, D2], F32, tag="x2sb")
                        ssb = wp.tile([128, 1], F32, tag="ssb")
                        nc.scalar.activation(x2sb[:], x2ps[:, :D2], Relu,
                                             accum_out=ssb[:])
                        nc.scalar.activation(
                            res[:, t:t + 1], ssb[:], Sigmoid,
                            bias=wb_rep[:, 1:2], scale=wb_rep[:, 0:1])

            # ---------------- layer 1 ----------------
            do_layer(x0, last=False)

            # x1sb -> x1loc (dst-tile layout back to row-major [NDST, D])
            if NFULL:
                nc.sync.dma_start(
                    x1loc[: NFULL * TILE, :].rearrange("(t r) f -> r t f", r=TILE),
                    x1sb[:, : NFULL * D].rearrange("p (t f) -> p t f", f=D),
                )
            if REM:
                nc.sync.dma_start(
                    x1loc[NFULL * TILE:, :],
                    x1sb[:REM, NFULL * D:(NFULL + 1) * D],
                )
            if cfg.no_cc:
                nc.sync.dma_start(x1full[:NDST, :], x1loc[:, :])
            else:
                nc.gpsimd.collective_compute(
                    "AllGather",
                    mybir.AluOpType.bypass,
                    replica_groups=[list(range(C))],
                    ins=[x1loc[:, :]],
                    outs=[x1full[:, :]],
                )

            # ---------------- layer 2 + head ----------------
            do_layer(x1full, last=True)

            if NFULL:
                nc.sync.dma_start(
                    outp[: NFULL * TILE, :].rearrange("(t r) o -> r (t o)", r=TILE),
                    res[:, :NFULL],
                )
            if REM:
                nc.sync.dma_start(
                    outp[NFULL * TILE:, :],
                    res[:REM, NFULL:NFULL + 1],
                )

    nc.finalize()
    return nc


_CACHE = {}


def _get_program(cfg, structure):
    key = (cfg.N, cfg.D, cfg.C, cfg.CH, cfg.BSZ, cfg.no_cc,
           structure["T"], structure["batches"], structure["group_tile"],
           structure["group_first"], structure["group_last"])
    if key not in _CACHE:
        _CACHE[key] = build_program(cfg, structure)
    return _CACHE[key]


OH_GROUPS = 16

# exposed for test.py to rerun with tracing without rebuilding
LAST_RUN = {}


def kernel(node_features, edge_src, edge_dst, W1, b1, W2, b2, Wd, bd,
           cfg=None, trace=False):
    cfg = cfg or Cfg(N=node_features.shape[0])
    structure, per_core = plan_edges(edge_src, edge_dst, cfg)
    nc = _get_program(cfg, structure)

    x0 = np.ascontiguousarray(np.asarray(node_features, dtype=np.float32))
    iota = np.tile(np.arange(128, dtype=np.float32), OH_GROUPS)[None, :].repeat(
        128, axis=0).copy()
    ident = np.eye(128, dtype=np.float32)
    ones1 = np.ones((1, 128), np.float32)
    wdbd = np.array([[np.asarray(Wd).reshape(-1)[0],
                      np.asarray(bd).reshape(-1)[0]]], np.float32)
    shared = dict(
        x0=x0,
        w1=np.ascontiguousarray(np.asarray(W1, np.float32)),
        b1=np.asarray(b1, np.float32).reshape(1, -1),
        w2=np.ascontiguousarray(np.asarray(W2, np.float32)),
        b2=np.asarray(b2, np.float32).reshape(1, -1),
        wdbd=wdbd,
        iota=iota,
        ident=ident,
        ones1=ones1,
    )
    in_maps = []
    for c in range(cfg.C):
        m = dict(shared)
        m.update(per_core[c])
        in_maps.append(m)

    core_ids = list(range(cfg.C))
    r = run_bass_kernel_spmd(nc, in_maps, core_ids, trace=trace)
    LAST_RUN["nc"] = nc
    LAST_RUN["in_maps"] = in_maps
    LAST_RUN["results"] = r
    out = np.concatenate([r.results[c]["out"] for c in range(cfg.C)], axis=0)
    return out



# revision 5
# speedup vs baseline: 1.1872x; 1.1872x over previous
"""Two-layer GraphConv (gather + segment-mean + linear + ReLU) x2 + sigmoid head,
distributed over 8 NeuronCores.

Sharding: destination nodes are partitioned across the 8 cores (12.5k each).
Host-side prep (pure index work): each core's edges are bucketed by
(src-chunk-of-25k, dst), each (chunk x dst-tile-of-128) run is padded to a
multiple of 128 with sentinel edges so all 8 cores share one SPMD program.

On device, per layer:
  - dma_gather fetches 256B fp32 source rows (int16 chunk-local indices)
  - gathered messages are cast fp32 -> bf16 on the scalar engine
  - one-hot matrices are built in bf16 on the vector engine by comparing an
    iota constant against per-edge relative-dst values
  - TensorE bf16 matmuls (lhsT=one-hot, rhs=msgs) segment-sum into fp32 PSUM,
    accumulated per dst-tile into an SBUF accumulator
  - scale by 1/deg (out bf16), PE-transpose, fused bf16 W+bias matmuls, ReLU
  - AllGather of x1 (fp32; layer-2 gather needs 256B rows) between the layers
  - layer-2 tail: ReLU with accumulated row-sum, sigmoid(scale*s+bias)
"""

import os
import sys

for _p in ("/opt/trn_rl_repo", "/opt/pypackages"):
    if _p not in sys.path and os.path.isdir(_p):
        sys.path.insert(0, _p)

import ml_dtypes
import numpy as np

from concourse import bacc, bass, mybir, tile
from concourse.bass_utils import run_bass_kernel_spmd

F32 = mybir.dt.float32
BF16 = mybir.dt.bfloat16
I16 = mybir.dt.int16

TILE = 128


def _cdiv(a, b):
    return (a + b - 1) // b


class Cfg:
    def __init__(self, N=100000, D=64, C=8, CH=25000, BSZ=2048, no_cc=False,
                 bufs=6):
        self.no_cc = no_cc
        assert N % C == 0 and N % CH == 0
        assert CH <= 32768  # int16 gather indices
        assert BSZ % 128 == 0
        self.N, self.D, self.C, self.CH, self.BSZ = N, D, C, CH, BSZ
        self.bufs = bufs
        self.NDST = N // C
        self.NT = _cdiv(self.NDST, TILE)
        self.NP = N // CH
        self.D2 = 32  # layer-2 output width


def plan_edges(edge_src, edge_dst, cfg):
    """Bucket/sort/pad edges per core; all cores share the quota structure."""
    src = np.asarray(edge_src).astype(np.int64)
    dst = np.asarray(edge_dst).astype(np.int64)
    C, CH, NT, NP, NDST = cfg.C, cfg.CH, cfg.NT, cfg.NP, cfg.NDST

    percore = []
    counts = []
    for c in range(C):
        m = (dst // NDST) == c
        s = src[m]
        dl = dst[m] - c * NDST
        p = s // CH
        o = np.lexsort((dl, p))
        s, dl, p = s[o], dl[o], p[o]
        t = dl >> 7
        cnt = np.bincount(p * NT + t, minlength=NP * NT).reshape(NP, NT)
        percore.append((s, dl, p, t))
        counts.append(cnt)

    quota = np.maximum.reduce(counts)
    quota = (quota + TILE - 1) // TILE * TILE  # pad runs to group multiples
    qflat = quota.reshape(-1)
    offs = np.concatenate([[0], np.cumsum(qflat)])
    T = int(offs[-1])
    offs_flat = offs[:-1].reshape(NP, NT)
    Lp = quota.sum(axis=1)

    # batches: per pass, chunks of BSZ stream positions (last one ragged)
    batches = []  # list of (pass, global_offset, nb)
    pass_base = np.concatenate([[0], np.cumsum(Lp)])
    for p in range(NP):
        off = 0
        while off < Lp[p]:
            nb = int(min(cfg.BSZ, Lp[p] - off))
            batches.append((p, int(pass_base[p] + off), nb))
            off += nb

    # group -> tile map + run boundary flags (shared across cores)
    NG = T // TILE
    group_tile = np.zeros(NG, np.int32)
    group_first = np.zeros(NG, bool)
    group_last = np.zeros(NG, bool)
    for p in range(NP):
        for t in range(NT):
            q = quota[p, t]
            if q == 0:
                continue
            g0 = offs_flat[p, t] // TILE
            g1 = g0 + q // TILE
            group_tile[g0:g1] = t
            group_first[g0] = True
            group_last[g1 - 1] = True

    per_core_arrays = []
    for c in range(C):
        s, dl, p, t = percore[c]
        key = p * NT + t
        first = np.searchsorted(key, np.arange(NP * NT), side="left")
        rank = np.arange(len(key)) - first[key]
        pos = offs_flat[p, t] + rank
        srcl = np.zeros(T, np.int16)
        drel = np.full(T, 200.0, np.float32)  # sentinel: never matches iota 0..127
        srcl[pos] = (s - p * CH).astype(np.int16)
        drel[pos] = (dl - (t << 7)).astype(np.float32)

        deg = np.bincount(dl, minlength=NDST).astype(np.float32)
        deg = np.maximum(deg, 1.0)
        degp = np.ones(NT * TILE, np.float32)
        degp[:NDST] = deg
        deg_arr = degp.reshape(NT, TILE).T.copy()  # [128, NT]

        idxw = np.tile(srcl.reshape(T // 16, 16).T, (8, 1)).copy()  # [128, T/16]
        drw = drel.reshape(T // TILE, TILE).T.copy()  # [128, T/128]
        per_core_arrays.append(dict(idxs=idxw, drel=drw, deg=deg_arr))

    structure = dict(
        T=T,
        NG=NG,
        batches=tuple(batches),
        group_tile=tuple(int(v) for v in group_tile),
        group_first=tuple(bool(v) for v in group_first),
        group_last=tuple(bool(v) for v in group_last),
    )
    return structure, per_core_arrays


def build_program(cfg, structure):
    N, D, C, CH, NT, NP = cfg.N, cfg.D, cfg.C, cfg.CH, cfg.NT, cfg.NP
    D2 = cfg.D2
    NDST = cfg.NDST
    T = structure["T"]
    batches = structure["batches"]
    group_tile = structure["group_tile"]
    group_first = structure["group_first"]
    group_last = structure["group_last"]
    OH_GROUPS = 16  # one-hot groups built per DVE op
    Relu = mybir.ActivationFunctionType.Relu
    Copy = mybir.ActivationFunctionType.Copy
    Sigmoid = mybir.ActivationFunctionType.Sigmoid

    nc = bacc.Bacc(None, target_bir_lowering=False, num_swdge_queues=4)
    x0 = nc.dram_tensor("x0", [N, D], F32, kind="ExternalInput")
    idxs_d = nc.dram_tensor("idxs", [128, T // 16], I16, kind="ExternalInput")
    drel_d = nc.dram_tensor("drel", [128, T // TILE], F32, kind="ExternalInput")
    deg_d = nc.dram_tensor("deg", [128, NT], F32, kind="ExternalInput")
    w1_d = nc.dram_tensor("w1", [D, D], BF16, kind="ExternalInput")
    b1_d = nc.dram_tensor("b1", [1, D], BF16, kind="ExternalInput")
    w2_d = nc.dram_tensor("w2", [D, D2], BF16, kind="ExternalInput")
    b2_d = nc.dram_tensor("b2", [1, D2], BF16, kind="ExternalInput")
    wdbd_d = nc.dram_tensor("wdbd", [1, 2], F32, kind="ExternalInput")
    iota_d = nc.dram_tensor("iota", [128, OH_GROUPS * TILE], F32, kind="ExternalInput")
    ident_d = nc.dram_tensor("ident", [128, 128], F32, kind="ExternalInput")
    ones_d = nc.dram_tensor("ones1", [1, 128], BF16, kind="ExternalInput")
    onesf_d = nc.dram_tensor("onesf", [1, 128], F32, kind="ExternalInput")
    outp = nc.dram_tensor("out", [NDST, 1], F32, kind="ExternalOutput")
    x1loc = nc.dram_tensor("x1loc", [NDST, D], F32)
    x1full = nc.dram_tensor("x1full", [N, D], F32, addr_space="Shared")

    NFULL = NDST // TILE  # full dst tiles
    REM = NDST - NFULL * TILE  # lanes in the last (partial) tile, 0 if none

    B = cfg.bufs

    with tile.TileContext(nc) as tc:
        with (
            tc.tile_pool(name="const", bufs=1) as cp,
            tc.tile_pool(name="work", bufs=B) as wp,
            tc.tile_pool(name="ohp", bufs=B) as ohp,
            tc.tile_pool(name="psacc", bufs=4, space="PSUM") as ps_acc,
            tc.tile_pool(name="pst", bufs=2, space="PSUM") as ps_t,
            tc.tile_pool(name="psm", bufs=2, space="PSUM") as ps_m,
        ):
            # ---- constants into SBUF ----
            iota_sb = cp.tile([128, OH_GROUPS * TILE], F32)
            nc.sync.dma_start(iota_sb[:], iota_d[:, :])
            identf_sb = cp.tile([128, 128], F32)
            nc.sync.dma_start(identf_sb[:], ident_d[:, :])
            ones_sb = cp.tile([1, 128], BF16)
            nc.sync.dma_start(ones_sb[:], ones_d[:, :])
            onesf_sb = cp.tile([1, 128], F32)
            nc.sync.dma_start(onesf_sb[:], onesf_d[:, :])
            w1_sb = cp.tile([D, D], BF16)
            nc.sync.dma_start(w1_sb[:], w1_d[:, :])
            b1_sb = cp.tile([1, D], BF16)
            nc.sync.dma_start(b1_sb[:], b1_d[:, :])
            w2_sb = cp.tile([D, D2], BF16)
            nc.sync.dma_start(w2_sb[:], w2_d[:, :])
            b2_sb = cp.tile([1, D2], BF16)
            nc.sync.dma_start(b2_sb[:], b2_d[:, :])
            wdbd_sb = cp.tile([1, 2], F32)
            nc.sync.dma_start(wdbd_sb[:], wdbd_d[:, :])
            deg_sb = cp.tile([128, NT], F32)
            nc.sync.dma_start(deg_sb[:], deg_d[:, :])

            rdeg = cp.tile([128, NT], F32)
            nc.vector.reciprocal(rdeg[:], deg_sb[:])

            def pe_fence(*aps):
                for ap in aps:
                    with tc.tile_critical():
                        nop = nc.tensor.nop(hint="dep").ins
                        nop.ins = [nc.tensor.lower_ap(ap)]

            # broadcast Wd/32 and bd across partitions via a K=1 matmul
            pe_fence(onesf_sb[:], wdbd_sb[:])
            wb_ps = ps_m.tile([128, 64], F32, tag="mm", name="wb_ps")
            nc.tensor.matmul(wb_ps[:, :2], lhsT=onesf_sb[:], rhs=wdbd_sb[:],
                             start=True, stop=True)
            wb_rep = cp.tile([128, 2], F32)
            nc.scalar.activation(wb_rep[:], wb_ps[:, :2], Copy)
            nc.vector.tensor_scalar_mul(wb_rep[:, 0:1], wb_rep[:, 0:1], 1.0 / 32.0)

            agg = cp.tile([128, NT * D], F32)
            x1sb = cp.tile([128, NT * D], F32)
            res = cp.tile([128, NT], F32)

            def do_layer(table, last):
                nc.vector.memset(agg[:], 0.0)
                cur_ps = [None]

                for bi, (p, boff, nb) in enumerate(batches):
                    ncol = nb // TILE
                    idx_t = wp.tile([128, nb // 16], I16, tag="idx")
                    nc.sync.dma_start(
                        idx_t[:], idxs_d[:, boff // 16:(boff + nb) // 16])
                    dr_t = wp.tile([128, ncol], F32, tag="dr")
                    nc.sync.dma_start(
                        dr_t[:], drel_d[:, boff // TILE:(boff + nb) // TILE])
                    msgs = wp.tile([128, ncol * D], F32, tag="msgs")
                    msgs3 = msgs[:].rearrange("p (c f) -> p c f", f=D)
                    nc.gpsimd.dma_gather(
                        msgs3,
                        table[p * CH:(p + 1) * CH, :],
                        idx_t[:],
                        nb,
                        nb,
                        D,
                        queue_num=bi % 4,
                    )
                    msgsb = wp.tile([128, ncol * D], BF16, tag="msgsb")
                    nc.scalar.activation(msgsb[:], msgs[:], Copy)
                    nsub = _cdiv(ncol, OH_GROUPS)
                    for sc in range(nsub):
                        gcols = min(OH_GROUPS, ncol - sc * OH_GROUPS)
                        m = gcols * TILE
                        oh = ohp.tile([128, OH_GROUPS * TILE], BF16, tag="oh")
                        in1 = (
                            dr_t[:, sc * OH_GROUPS: sc * OH_GROUPS + gcols]
                            .rearrange("p (g o) -> p g o", o=1)
                            .to_broadcast([128, gcols, TILE])
                        )
                        nc.vector.tensor_tensor(
                            out=oh[:, :m],
                            in0=iota_sb[:, :m],
                            in1=in1,
                            op=mybir.AluOpType.is_equal,
                        )
                        pe_fence(oh[:, :m], msgsb[:])
                        for g in range(gcols):
                            gg = boff // TILE + sc * OH_GROUPS + g
                            t = group_tile[gg]
                            if group_first[gg]:
                                cur_ps[0] = ps_acc.tile(
                                    [128, D], F32, tag="acc", name="accps")
                            nc.tensor.matmul(
                                cur_ps[0][:],
                                lhsT=oh[:, g * TILE:(g + 1) * TILE],
                                rhs=msgsb[:, (sc * OH_GROUPS + g) * D:
                                          (sc * OH_GROUPS + g + 1) * D],
                                start=group_first[gg],
                                stop=group_last[gg],
                            )
                            if group_last[gg]:
                                nc.vector.tensor_add(
                                    agg[:, t * D:(t + 1) * D],
                                    agg[:, t * D:(t + 1) * D],
                                    cur_ps[0][:],
                                )

                for t in range(NT):
                    scaled = wp.tile([128, D], F32, tag="scaled")
                    nc.vector.tensor_scalar_mul(
                        scaled[:], agg[:, t * D:(t + 1) * D], rdeg[:, t:t + 1])
                    pe_fence(scaled[:], identf_sb[:])
                    tps = ps_t.tile([D, 128], F32, tag="tps")
                    nc.tensor.transpose(tps[:], scaled[:], identf_sb[:])
                    aggT = wp.tile([D, 128], BF16, tag="aggT")
                    nc.scalar.activation(aggT[:], tps[:], Copy)
                    if not last:
                        pe_fence(aggT[:], w1_sb[:], ones_sb[:], b1_sb[:])
                        x1ps = ps_m.tile([128, D], F32, tag="mm", name="x1ps")
                        nc.tensor.matmul(x1ps[:], lhsT=aggT[:], rhs=w1_sb[:],
                                         start=True, stop=False)
                        nc.tensor.matmul(x1ps[:], lhsT=ones_sb[:], rhs=b1_sb[:],
                                         start=False, stop=True)
                        nc.scalar.activation(
                            x1sb[:, t * D:(t + 1) * D], x1ps[:], Relu)
                    else:
                        pe_fence(aggT[:], w2_sb[:], ones_sb[:], b2_sb[:])
                        x2ps = ps_m.tile([128, D], F32, tag="mm", name="x2ps")
                        nc.tensor.matmul(x2ps[:, :D2], lhsT=aggT[:], rhs=w2_sb[:],
                                         start=True, stop=False)
                        nc.tensor.matmul(x2ps[:, :D2], lhsT=ones_sb[:], rhs=b2_sb[:],
                                         start=False, stop=True)
                        x2sb = wp.tile([128, D2], F32, tag="x2sb")
                        ssb = wp.tile([128, 1], F32, tag="ssb")
                        nc.scalar.activation(x2sb[:], x2ps[:, :D2], Relu,
                                             accum_out=ssb[:])
                        nc.scalar.activation(
                            res[:, t:t + 1], ssb[:], Sigmoid,
                            bias=wb_rep[:, 1:2], scale=wb_rep[:, 0:1])

            # ---------------- layer 1 ----------------
            do_layer(x0, last=False)

            # x1sb -> x1loc (dst-tile layout back to row-major [NDST, D])
            if NFULL:
                nc.sync.dma_start(
                    x1loc[: NFULL * TILE, :].rearrange("(t r) f -> r t f", r=TILE),
                    x1sb[:, : NFULL * D].rearrange("p (t f) -> p t f", f=D),
                )
            if REM:
                nc.sync.dma_start(
                    x1loc[NFULL * TILE:, :],
                    x1sb[:REM, NFULL * D:(NFULL + 1) * D],
                )
            if cfg.no_cc:
                nc.sync.dma_start(x1full[:NDST, :], x1loc[:, :])
            else:
                nc.gpsimd.collective_compute(
                    "AllGather",
                    mybir.AluOpType.bypass,
                    replica_groups=[list(range(C))],
                    ins=[x1loc[:, :]],
                    outs=[x1full[:, :]],
                )

            # ---------------- layer 2 + head ----------------
            do_layer(x1full, last=True)

            if NFULL:
                nc.sync.dma_start(
                    outp[: NFULL * TILE, :].rearrange("(t r) o -> r (t o)", r=TILE),
                    res[:, :NFULL],
                )
            if REM:
                nc.sync.dma_start(
                    outp[NFULL * TILE:, :],
                    res[:REM, NFULL:NFULL + 1],
                )

    nc.finalize()
    return nc


_CACHE = {}


def _get_program(cfg, structure):
    key = (cfg.N, cfg.D, cfg.C, cfg.CH, cfg.BSZ, cfg.no_cc, cfg.bufs,
           structure["T"], structure["batches"], structure["group_tile"],
           structure["group_first"], structure["group_last"])
    if key not in _CACHE:
        _CACHE[key] = build_program(cfg, structure)
    return _CACHE[key]


OH_GROUPS = 16

# exposed for test.py to rerun with tracing without rebuilding
LAST_RUN = {}


def kernel(node_features, edge_src, edge_dst, W1, b1, W2, b2, Wd, bd,
           cfg=None, trace=False):
    cfg = cfg or Cfg(N=node_features.shape[0])
    structure, per_core = plan_edges(edge_src, edge_dst, cfg)
    nc = _get_program(cfg, structure)

    bf16 = ml_dtypes.bfloat16
    x0 = np.ascontiguousarray(np.asarray(node_features, dtype=np.float32))
    iota = np.tile(np.arange(128, dtype=np.float32), OH_GROUPS)[None, :].repeat(
        128, axis=0).copy()
    ident = np.eye(128, dtype=np.float32)
    ones1 = np.ones((1, 128), bf16)
    onesf = np.ones((1, 128), np.float32)
    wdbd = np.array([[np.asarray(Wd).reshape(-1)[0],
                      np.asarray(bd).reshape(-1)[0]]], np.float32)
    shared = dict(
        x0=x0,
        w1=np.ascontiguousarray(np.asarray(W1, np.float32).astype(bf16)),
        b1=np.asarray(b1, np.float32).reshape(1, -1).astype(bf16),
        w2=np.ascontiguousarray(np.asarray(W2, np.float32).astype(bf16)),
        b2=np.asarray(b2, np.float32).reshape(1, -1).astype(bf16),
        wdbd=wdbd,
        iota=iota,
        ident=ident,
        ones1=ones1,
        onesf=onesf,
    )
    in_maps = []
    for c in range(cfg.C):
        m = dict(shared)
        m.update(per_core[c])
        in_maps.append(m)

    core_ids = list(range(cfg.C))
    r = run_bass_kernel_spmd(nc, in_maps, core_ids, trace=trace)
    LAST_RUN["nc"] = nc
    LAST_RUN["in_maps"] = in_maps
    LAST_RUN["results"] = r
    out = np.concatenate([r.results[c]["out"] for c in range(cfg.C)], axis=0)
    return out
